# revision 5
# baseline (speedup 1.0000x reference)
"""Multi-head attention Trainium2 Bass kernel.

Problem: B=8, S=1024, D=768, H=12, head_dim=64; per-head block-diagonal QKV
projections + softmax attention (no 1/sqrt(hd) scaling).

Sharding: data-parallel over batch — one batch element per NeuronCore (8 cores).
No collectives; host scatters inputs / gathers outputs.

Per-core dataflow (all heads, channel-on-partition "transposed" layouts):
  x [S,D] --PE-transpose--> xT (f32r)  [6 blocks of 128 ch x S]
  qT/kT = W.T @ xT + b     (f32r matmuls, head pairs row+col tiled on the PE)
  v natural = xT_bf16.T @ Wv_bf16 (+ rank-1 bias with a ones column appended ->
      v~ [t,65] whose col 64 == 1)
  LT = K Q^T per head      ([t,s] layout, f32r, head pairs row-tiled)
  E = exp(LT)              (ScalarE, one [128,2048] op per pair/t-tile -> bf16)
  UT = v~^T @ E            (bf16; rows 0..63 = unnormalized out^T, row 64 =
      softmax denominator via the ones column)
  out = transpose(UT) * recip(denom)  (PE transpose + DVE per-s-tile scale)
"""
import numpy as np

S = 1024
D = 768
H = 12
HD = 64
NPAIR = H // 2   # 6
NCORES = 8
ST = S // 128    # 8 s-tiles
TT = S // 128    # 8 t-tiles

_CACHE = {}


def _build():
    import concourse.bacc as bacc
    import concourse.mybir as mybir
    import concourse.tile as tile
    from concourse.masks import make_identity

    f32 = mybir.dt.float32
    f32r = mybir.dt.float32r
    bf16 = mybir.dt.bfloat16

    nc = bacc.Bacc("TRN2", target_bir_lowering=False, debug=False,
                   num_devices=NCORES)
    x = nc.declare_dram_parameter("x", [S, D], f32, isOutput=False)
    Wq = nc.declare_dram_parameter("Wq", [H, HD, HD], f32, isOutput=False)
    bq = nc.declare_dram_parameter("bq", [H, HD], f32, isOutput=False)
    Wk = nc.declare_dram_parameter("Wk", [H, HD, HD], f32, isOutput=False)
    bk = nc.declare_dram_parameter("bk", [H, HD], f32, isOutput=False)
    Wv = nc.declare_dram_parameter("Wv", [H, HD, HD], f32, isOutput=False)
    bv = nc.declare_dram_parameter("bv", [H, HD], f32, isOutput=False)
    out = nc.declare_dram_parameter("out", [S, D], f32, isOutput=True)

    with tile.TileContext(nc) as tc:
        import contextlib
        with contextlib.ExitStack() as ctx:
            singles = ctx.enter_context(tc.tile_pool(name="singles", bufs=1))
            # persistent SBUF
            qkv_pool = ctx.enter_context(tc.tile_pool(name="qkv", bufs=1))
            stage_pool = ctx.enter_context(tc.tile_pool(name="stage", bufs=1))

            # ---- constants ----
            ident_f = singles.tile([128, 128], f32)
            make_identity(nc, ident_f)
            ident_r = singles.tile([128, 128], f32r)
            nc.vector.tensor_copy(ident_r, ident_f)
            ones_bf = singles.tile([1, 128], bf16)
            nc.vector.memset(ones_bf, 1.0)

            # ---- weights: [128, NPAIR, HD]; parts 0:64 = even head (d-dim),
            #      64:128 = odd head ----
            def load_w_pair(w_dram, dtype, ncols, tag):
                raw = singles.tile([128, NPAIR, HD], f32, tag="wraw",
                                   name=f"raw_{tag}")
                nc.sync.dma_start(
                    out=raw[0:64, :, :],
                    in_=w_dram[0:H:2, :, :].rearrange("h d e -> d h e"))
                nc.sync.dma_start(
                    out=raw[64:128, :, :],
                    in_=w_dram[1:H:2, :, :].rearrange("h d e -> d h e"))
                wt = singles.tile([128, NPAIR, ncols], dtype, tag=tag,
                                  name=tag)
                if ncols != HD:
                    nc.vector.memset(wt, 0.0)
                nc.vector.tensor_copy(wt[:, :, 0:HD], raw)
                return wt

            wq_r = load_w_pair(Wq, f32r, HD, "wqr")
            wk_r = load_w_pair(Wk, f32r, HD, "wkr")
            wv_bf = load_w_pair(Wv, bf16, HD + 1, "wvbf")

            # ---- biases ----
            # bq/bk: [128, NPAIR] f32; partition = e + 64*(h%2), free = pair
            def load_b_pair(b_dram, tag):
                bt = singles.tile([128, NPAIR], f32, tag=tag, name=tag)
                nc.sync.dma_start(out=bt[0:64, :],
                                  in_=b_dram[0:H:2, :].rearrange("h e -> e h"))
                nc.sync.dma_start(out=bt[64:128, :],
                                  in_=b_dram[1:H:2, :].rearrange("h e -> e h"))
                return bt

            bq_sb = load_b_pair(bq, "bqsb")
            bk_sb = load_b_pair(bk, "bksb")
            # bv augmented: [1, H, HD+1] bf16, col HD = 1.0 (ones column)
            bv_raw = singles.tile([1, H, HD], f32, tag="bvraw")
            nc.sync.dma_start(out=bv_raw, in_=bv[None, :, :])
            bv_bf = singles.tile([1, H, HD + 1], bf16)
            nc.vector.memset(bv_bf, 1.0)
            nc.vector.tensor_copy(bv_bf[:, :, 0:HD], bv_raw)

            # ---- persistent activation buffers ----
            xT_r = [qkv_pool.tile([128, S], f32r, tag=f"xT{i}", name=f"xT{i}")
                    for i in range(6)]
            xT_bf = [qkv_pool.tile([128, S], bf16, tag=f"xTb{i}", name=f"xTb{i}")
                     for i in range(6)]
            qT = [qkv_pool.tile([128, S], f32r, tag=f"qT{p}", name=f"qT{p}")
                  for p in range(NPAIR)]
            kT = [qkv_pool.tile([128, S], f32r, tag=f"kT{p}", name=f"kT{p}")
                  for p in range(NPAIR)]
            v_bf = [qkv_pool.tile([128, TT, HD + 1], bf16, tag=f"v{h}", name=f"v{h}")
                    for h in range(H)]
            staging = [stage_pool.tile([128, D], f32, tag=f"stg{st}", name=f"stg{st}")
                       for st in range(ST)]

            # ================= phase 0: load + transpose x =================
            with tc.tile_pool(name="xload", bufs=2) as xload, \
                 tc.tile_pool(name="tp_ps", bufs=4, space="PSUM") as tp_ps:
                for st in range(ST):
                    x_f = xload.tile([128, D], f32, tag="xf")
                    nc.sync.dma_start(out=x_f, in_=x[st * 128:(st + 1) * 128, :])
                    x_r = xload.tile([128, D], f32r, tag="xr")
                    nc.vector.tensor_copy(x_r, x_f)
                    for db in range(6):
                        tp = tp_ps.tile([128, 128], f32r, tag="tp")
                        nc.tensor.transpose(
                            tp, x_r[:, db * 128:(db + 1) * 128], ident_r)
                        nc.vector.tensor_copy(
                            xT_r[db][:, st * 128:(st + 1) * 128], tp)
                for db in range(6):
                    nc.vector.tensor_copy(xT_bf[db], xT_r[db])

            # ================= phase 1: projections =================
            with tc.tile_pool(name="p1_ps", bufs=2, space="PSUM") as p1_ps:
                for p in range(NPAIR):
                    # q, k: two [64, S] psums, row-tiled (0,0)/(64,0);
                    # (64,64) diagonal placement is a broken PE quadrant.
                    for (wt, bt, dst) in ((wq_r, bq_sb, qT[p]),
                                          (wk_r, bk_sb, kT[p])):
                        psA = p1_ps.tile([64, S], f32, tag="proj", name="psA")
                        psB = p1_ps.tile([64, S], f32, tag="proj", name="psB")
                        for sp in range(2):
                            sl = slice(sp * 512, (sp + 1) * 512)
                            nc.tensor.matmul(
                                psA[:, sl], wt[0:64, p, :],
                                xT_r[p][0:64, sl], start=True, stop=True)
                            nc.tensor.matmul(
                                psB[:, sl], wt[64:128, p, :],
                                xT_r[p][64:128, sl], start=True, stop=True)
                        nc.vector.tensor_scalar_add(
                            dst[0:64, :], psA, bt[0:64, p:p + 1])
                        nc.vector.tensor_scalar_add(
                            dst[64:128, :], psB, bt[64:128, p:p + 1])
                    # v natural for both heads of the pair (bf16, N=65)
                    for hh in range(2):
                        h = 2 * p + hh
                        base = 64 * hh
                        for half in range(2):  # t-tiles 0-3 / 4-7
                            pv = p1_ps.tile([128, 4, HD + 1], f32, tag="proj")
                            for j in range(4):
                                tt = half * 4 + j
                                nc.tensor.matmul(
                                    pv[:, j, :],
                                    xT_bf[p][base:base + 64,
                                             tt * 128:(tt + 1) * 128],
                                    wv_bf[base:base + 64, p, :],
                                    start=True, stop=False)
                                nc.tensor.matmul(
                                    pv[:, j, :], ones_bf, bv_bf[:, h, :],
                                    start=False, stop=True)
                            nc.vector.tensor_copy(
                                v_bf[h][:, half * 4:(half + 1) * 4, :], pv)

            # ================= phase 2: attention =================
            with tc.tile_pool(name="lt_ps", bufs=1, space="PSUM") as lt_ps, \
                 tc.tile_pool(name="ut_ps", bufs=2, space="PSUM") as ut_ps, \
                 tc.tile_pool(name="ot_ps", bufs=2, space="PSUM") as ot_ps, \
                 tc.tile_pool(name="et_sb", bufs=12) as et_sb, \
                 tc.tile_pool(name="tail_sb", bufs=4) as tail_sb:
                for p in range(NPAIR):
                    # logits^T for the pair: [t, sA | sB], exp -> bf16
                    ET = [et_sb.tile([128, 2048], bf16, tag="et", name=f"et{p}_{i}")
                          for i in range(TT)]
                    for tt in range(TT):
                        lt = lt_ps.tile([128, 2048], f32, tag="lt")
                        tsl = slice(tt * 128, (tt + 1) * 128)
                        for sp in range(2):
                            nc.tensor.matmul(
                                lt[:, sp * 512:(sp + 1) * 512],
                                kT[p][0:64, tsl],
                                qT[p][0:64, sp * 512:(sp + 1) * 512],
                                start=True, stop=True)
                            nc.tensor.matmul(
                                lt[:, 1024 + sp * 512:1024 + (sp + 1) * 512],
                                kT[p][64:128, tsl],
                                qT[p][64:128, sp * 512:(sp + 1) * 512],
                                start=True, stop=True)
                        nc.scalar.activation(
                            ET[tt], lt, mybir.ActivationFunctionType.Exp)
                    # AV per head, per 512-wide s-span
                    for hh in range(2):
                        h = 2 * p + hh
                        for sp2 in range(2):
                            ut = ut_ps.tile([HD + 1, 512], f32, tag="ut")
                            off = hh * 1024 + sp2 * 512
                            for tt in range(TT):
                                nc.tensor.matmul(
                                    ut, v_bf[h][:, tt, :],
                                    ET[tt][:, off:off + 512],
                                    start=(tt == 0), stop=(tt == TT - 1))
                            u_sb = tail_sb.tile([HD + 1, 512], f32, tag="usb")
                            nc.vector.tensor_copy(u_sb, ut)
                            for j in range(4):
                                st = sp2 * 4 + j
                                ot = ot_ps.tile([128, HD + 1], f32, tag="ot")
                                nc.tensor.transpose(
                                    ot, u_sb[:, j * 128:(j + 1) * 128],
                                    ident_f[0:HD + 1, 0:HD + 1])
                                rc = tail_sb.tile([128, 1], f32, tag="rc")
                                nc.vector.reciprocal(rc, ot[:, HD:HD + 1])
                                nc.vector.tensor_scalar_mul(
                                    staging[st][:, h * HD:(h + 1) * HD],
                                    ot[:, 0:HD], rc)

            # ================= phase 3: write out =================
            for st in range(ST):
                nc.sync.dma_start(
                    out=out[st * 128:(st + 1) * 128, :], in_=staging[st])

    nc.compile()
    return nc


def _get_nc():
    if "nc" not in _CACHE:
        _CACHE["nc"] = _build()
    return _CACHE["nc"]


def kernel(**inputs) -> np.ndarray:
    from concourse.bass_utils import run_bass_kernel_spmd

    nc = _get_nc()
    seq = np.ascontiguousarray(np.asarray(inputs["sequences"], dtype=np.float32))
    common = {
        k: np.ascontiguousarray(np.asarray(inputs[k], dtype=np.float32))
        for k in ("Wq", "bq", "Wk", "bk", "Wv", "bv")
    }
    in_maps = [dict(common, x=seq[b]) for b in range(NCORES)]
    res = run_bass_kernel_spmd(nc, in_maps, list(range(NCORES)))
    return np.stack([res.results[b]["out"] for b in range(NCORES)], axis=0)


# revision 36
# speedup vs baseline: 1.9836x; 1.9836x over previous
"""Multi-head attention Trainium2 Bass kernel.

Problem: B=8, S=1024, D=768, H=12, head_dim=64; per-head block-diagonal QKV
projections + softmax attention (no 1/sqrt(hd) scaling).

Sharding: data-parallel over batch — one batch element per NeuronCore (8
cores). No collectives; host scatters inputs / gathers outputs.

Per-core dataflow (channel-on-partition "transposed" layouts; heads processed
in pairs p = (2p, 2p+1) matching 128-channel blocks of the embedding dim):
  x [S,D] --PE-transpose--> xT_r (f32r) / xT_bf (bf16)
  qT/kT = W.T @ xT + b      (f32r matmuls, pair row-tiled (0,0)/(64,0))
  v~ = x @ Wv | rank-1 bias update appends a ones column    ([t, 65] bf16)
  LT = K Q^T                ([t,s] layout, f32r); [128,1024] psum tiles,
                            bufs=2 so the next tile's matmuls overlap exp
  E^T = exp(LT)             (one ScalarE op per lt tile -> bf16)
  O = E @ v~ per (pair, s-tile): lhsT = E^T chunk (stationary), rhs = v~;
      psum [128(s), 130] = both heads; cols 64/129 = softmax denominators
  out = O * recip(denom)    (one batched DVE mul per group via 3-dim APs)

Scheduling: ScalarE is the bottleneck (~100us of exp); everything else is
interleaved into the exp ladder as filler so it never starves: transposes +
q/k proj of pair P ride in ladders < P, v-projections of pair p in ladder p,
attention-weighted-V of pair p-1 in ladder p, and the output DMA for head
pairs 0-4 of each s-tile overlaps the last ladder.
"""
import numpy as np

S = 1024
D = 768
H = 12
HD = 64
NPAIR = H // 2   # 6
NCORES = 8
ST = S // 128    # 8 s-tiles
TT = S // 128    # 8 t-tiles

_CACHE = {}


def _build():
    import contextlib
    import concourse.bacc as bacc
    import concourse.mybir as mybir
    import concourse.tile as tile
    from concourse.masks import make_identity

    f32 = mybir.dt.float32
    f32r = mybir.dt.float32r
    bf16 = mybir.dt.bfloat16
    Exp = mybir.ActivationFunctionType.Exp

    nc = bacc.Bacc("TRN2", target_bir_lowering=False, debug=False,
                   num_devices=NCORES)
    x = nc.declare_dram_parameter("x", [S, D], f32, isOutput=False)
    Wq = nc.declare_dram_parameter("Wq", [H, HD, HD], f32, isOutput=False)
    bq = nc.declare_dram_parameter("bq", [H, HD], f32, isOutput=False)
    Wk = nc.declare_dram_parameter("Wk", [H, HD, HD], f32, isOutput=False)
    bk = nc.declare_dram_parameter("bk", [H, HD], f32, isOutput=False)
    Wv = nc.declare_dram_parameter("Wv", [H, HD, HD], f32, isOutput=False)
    bv = nc.declare_dram_parameter("bv", [H, HD], f32, isOutput=False)
    out = nc.declare_dram_parameter("out", [S, D], f32, isOutput=True)

    with tile.TileContext(nc) as tc, contextlib.ExitStack() as ctx:
        singles = ctx.enter_context(tc.tile_pool(name="singles", bufs=1))
        per = ctx.enter_context(tc.tile_pool(name="per", bufs=1))
        qk_pool = ctx.enter_context(tc.tile_pool(name="qk", bufs=4))
        small_sb = ctx.enter_context(tc.tile_pool(name="small_sb", bufs=4))
        xload = ctx.enter_context(tc.tile_pool(name="xload", bufs=12))
        et_pool = ctx.enter_context(tc.tile_pool(name="et", bufs=36))
        # PSUM budget (8 banks): sp 2 + lt 2x2 + o 2x1 = 8
        sp_ps = ctx.enter_context(
            tc.tile_pool(name="sp_ps", bufs=2, space="PSUM"))
        lt_ps = ctx.enter_context(
            tc.tile_pool(name="lt_ps", bufs=2, space="PSUM"))
        o_ps = ctx.enter_context(
            tc.tile_pool(name="o_ps", bufs=2, space="PSUM"))

        # ---- persistent activations ----
        xT_r = [per.tile([128, S], f32r, tag=f"xT{i}", name=f"xT{i}")
                for i in range(6)]
        xT_bf = [per.tile([128, S], bf16, tag=f"xTb{i}", name=f"xTb{i}")
                 for i in range(6)]
        v_bf = [per.tile([128, TT, HD + 1], bf16, tag=f"v{h}", name=f"v{h}")
                for h in range(H)]
        staging = [per.tile([128, D], f32, tag=f"stg{i}", name=f"stg{i}")
                   for i in range(ST)]
        qT = {}
        kT = {}

        ident_f = singles.tile([128, 128], f32)
        make_identity(nc, ident_f)
        ones_bf = singles.tile([1, 128], bf16)
        nc.vector.memset(ones_bf, 1.0)

        def emit_transposes(p):
            """x block p -> xT_r[p]: per-block [128,128] chunk DMAs feeding
            two PE transposes per psum tile + one DVE copyback."""
            csl = slice(p * 128, (p + 1) * 128)
            for st2 in range(ST // 2):
                tp = sp_ps.tile([128, 256], f32, tag="sps", name="tp")
                for j in range(2):
                    st = st2 * 2 + j
                    xc = xload.tile([128, 128], f32, tag="xf",
                                    name=f"xc{p}_{st}")
                    nc.sync.dma_start(
                        out=xc, in_=x[st * 128:(st + 1) * 128, csl])
                    nc.tensor.transpose(
                        tp[:, j * 128:(j + 1) * 128], xc, ident_f)
                nc.vector.tensor_copy(
                    xT_r[p][:, st2 * 256:(st2 + 1) * 256], tp)

        emit_transposes(0)

        def load_w_pair(w_dram, dtype, ncols, tag):
            raw = singles.tile([128, NPAIR, HD], f32, tag="wraw",
                               name=f"raw_{tag}")
            nc.sync.dma_start(
                out=raw[0:64, :, :],
                in_=w_dram[0:H:2, :, :].rearrange("h d e -> d h e"))
            nc.sync.dma_start(
                out=raw[64:128, :, :],
                in_=w_dram[1:H:2, :, :].rearrange("h d e -> d h e"))
            wt = singles.tile([128, NPAIR, ncols], dtype, tag=tag, name=tag)
            if ncols != HD:
                nc.vector.memset(wt, 0.0)
            nc.vector.tensor_copy(wt[:, :, 0:HD], raw)
            return wt

        def load_b_pair(b_dram, tag):
            bt = singles.tile([128, NPAIR], f32, tag=tag, name=tag)
            nc.sync.dma_start(out=bt[0:64, :],
                              in_=b_dram[0:H:2, :].rearrange("h e -> e h"))
            nc.sync.dma_start(out=bt[64:128, :],
                              in_=b_dram[1:H:2, :].rearrange("h e -> e h"))
            return bt

        wq_r = load_w_pair(Wq, f32r, HD, "wqr")
        wk_r = load_w_pair(Wk, f32r, HD, "wkr")
        bq_sb = load_b_pair(bq, "bqsb")
        bk_sb = load_b_pair(bk, "bksb")
        wv_bf = load_w_pair(Wv, bf16, HD + 1, "wvbf")
        bv_raw = singles.tile([1, H, HD], f32, tag="bvraw")
        nc.sync.dma_start(out=bv_raw, in_=bv[None, :, :])
        bv_bf = singles.tile([1, H, HD + 1], bf16)
        nc.vector.memset(bv_bf, 1.0)
        nc.vector.tensor_copy(bv_bf[:, :, 0:HD], bv_raw)

        def emit_proj_qk(p):
            qT[p] = qk_pool.tile([128, S], f32r, tag="qT", name=f"qT{p}")
            kT[p] = qk_pool.tile([128, S], f32r, tag="kT", name=f"kT{p}")
            for (wt, bt, dst) in ((wq_r, bq_sb, qT[p]), (wk_r, bk_sb, kT[p])):
                for sp in range(2):
                    sl = slice(sp * 512, (sp + 1) * 512)
                    psA = sp_ps.tile([64, 512], f32, tag="sps", name="psA")
                    psB = sp_ps.tile([64, 512], f32, tag="sps", name="psB")
                    nc.tensor.matmul(psA, wt[0:64, p, :], xT_r[p][0:64, sl],
                                     start=True, stop=True)
                    nc.tensor.matmul(psB, wt[64:128, p, :],
                                     xT_r[p][64:128, sl],
                                     start=True, stop=True)
                    nc.vector.tensor_scalar_add(
                        dst[0:64, sl], psA, bt[0:64, p:p + 1])
                    nc.vector.tensor_scalar_add(
                        dst[64:128, sl], psB, bt[64:128, p:p + 1])

        def emit_prep(p):
            emit_transposes(p)
            emit_proj_qk(p)

        def emit_v(p):
            """bf16 cast of block p + v projections for both heads."""
            nc.vector.tensor_copy(xT_bf[p], xT_r[p])
            for hh in range(2):
                h = 2 * p + hh
                base = 64 * hh
                for half in range(2):
                    pv = sp_ps.tile([128, 4, HD + 1], f32, tag="sps",
                                    name="pv")
                    for j in range(4):
                        tt = half * 4 + j
                        nc.tensor.matmul(
                            pv[:, j, :],
                            xT_bf[p][base:base + 64,
                                     tt * 128:(tt + 1) * 128],
                            wv_bf[base:base + 64, p, :],
                            start=True, stop=False)
                        nc.tensor.matmul(
                            pv[:, j, :], ones_bf, bv_bf[:, h, :],
                            start=False, stop=True)
                    nc.vector.tensor_copy(
                        v_bf[h][:, half * 4:(half + 1) * 4, :], pv)

        ET = {}

        def emit_av_group(p, st):
            """O for both heads of pair p at s-tile st: [128, 130] psum;
            cols 64/129 hold the softmax denominators. Runs at mid priority
            (above other filler, below the ladder) so the ET tiles of pair p
            release before pair p+1's ladder needs the slots."""
            _emit_av_group(p, st)

        def _emit_av_group(p, st):
            po = o_ps.tile([128, 2 * (HD + 1)], f32, tag="po", name="po")
            for hh in range(2):
                h = 2 * p + hh
                osl = slice(hh * (HD + 1), (hh + 1) * (HD + 1))
                for tt in range(TT):
                    nc.tensor.matmul(
                        po[:, osl],
                        ET[p][2 * tt + hh][:, st * 128:(st + 1) * 128],
                        v_bf[h][:, tt, :],
                        start=(tt == 0), stop=(tt == TT - 1))
            rc = small_sb.tile([128, 2], f32, tag="rc", name="rc")
            po3 = po.rearrange("a (h e) -> a h e", e=HD + 1)
            nc.vector.reciprocal(rc, po3[:, :, HD])
            nc.vector.tensor_tensor(
                out=staging[st].rearrange("a (h e) -> a h e", e=HD)[
                    :, 2 * p:2 * p + 2, :],
                in0=po3[:, :, 0:HD],
                in1=rc.rearrange("a (h o) -> a h o", o=1).to_broadcast(
                    (128, 2, HD)),
                op=mybir.AluOpType.mult)

        def emit_av_head(p, hh, st):
            """Single-head AV + normalize (used to drain the last pair)."""
            h = 2 * p + hh
            po = o_ps.tile([128, HD + 1], f32, tag="po", name="po")
            for tt in range(TT):
                nc.tensor.matmul(
                    po, ET[p][2 * tt + hh][:, st * 128:(st + 1) * 128],
                    v_bf[h][:, tt, :],
                    start=(tt == 0), stop=(tt == TT - 1))
            rc = small_sb.tile([128, 1], f32, tag="rc", name="rc")
            nc.vector.reciprocal(rc, po[:, HD:HD + 1])
            nc.vector.tensor_scalar_mul(
                staging[st][:, h * HD:(h + 1) * HD], po[:, 0:HD], rc)

        def emit_ladder(p, filler, hh_major=False):
            """LT + exp ladder for pair p; ET tile index = 2*tt + hh.
            hh_major orders all head-0 exps first so that head-0's AV can
            overlap the head-1 exps (used for the last pair)."""
            ET[p] = [None] * (2 * TT)
            if hh_major:
                units = [(tt, hh) for hh in range(2) for tt in range(TT)]
            else:
                units = [(tt, hh) for tt in range(TT) for hh in range(2)]
            for tt, hh in units:
                tsl = slice(tt * 128, (tt + 1) * 128)
                if True:
                    rsl = slice(hh * 64, hh * 64 + 64)
                    with tc.high_priority(offset=400):
                        lt = lt_ps.tile([128, 1024], f32, tag="lt",
                                        name="lt")
                        for sp in range(2):
                            ssl = slice(sp * 512, (sp + 1) * 512)
                            nc.tensor.matmul(lt[:, ssl], kT[p][rsl, tsl],
                                             qT[p][rsl, ssl],
                                             start=True, stop=True)
                        et = et_pool.tile([128, 1024], bf16, tag="et",
                                          name=f"et{p}_{2 * tt + hh}")
                        ET[p][2 * tt + hh] = et
                        nc.scalar.activation(et, lt, Exp)
                    if filler:
                        filler.pop(0)()
            while filler:
                filler.pop(0)()
            if p - 1 in ET:
                del ET[p - 1]

        emit_proj_qk(0)
        # filler plans per ladder (see module docstring)
        plans = {
            0: [lambda: emit_prep(1), lambda: emit_v(0),
                lambda: emit_prep(2), lambda: emit_prep(3)],
            1: [lambda: emit_prep(4), lambda: emit_v(1)],
            2: [lambda: emit_prep(5), lambda: emit_v(2)],
            3: [lambda: emit_v(3)],
            4: [lambda: emit_v(4)],
            5: [lambda: emit_v(5)],
        }
        for p in range(NPAIR):
            filler = list(plans[p])
            # delay AV fillers to mid-ladder: their matmuls wait on the
            # previous pair's last exp and would head-block the in-order PE
            # stream if scheduled early
            while len(filler) < 5:
                filler.append(lambda: None)
            if p >= 1:
                for st in range(ST):
                    filler.append(lambda q=p - 1, s=st: emit_av_group(q, s))
                    if p == NPAIR - 1:
                        # pairs 0-4 of this s-tile are final: overlap the
                        # bulk of the output writeback with the last ladder
                        filler.append(lambda s=st: nc.sync.dma_start(
                            out=out[s * 128:(s + 1) * 128, 0:640],
                            in_=staging[s][:, 0:640]))
            if p == NPAIR - 1:
                # last pair: head-0 exps first, then while head-1 exps run,
                # head-0's AV groups drain as trailing filler
                for st in range(ST):
                    filler.append(
                        lambda s=st: emit_av_head(NPAIR - 1, 0, s))
                emit_ladder(p, filler, hh_major=True)
            else:
                emit_ladder(p, filler)
        for st in range(ST):
            emit_av_head(NPAIR - 1, 1, st)
            nc.sync.dma_start(
                out=out[st * 128:(st + 1) * 128, 640:768],
                in_=staging[st][:, 640:768])

    nc.compile()
    return nc


def _get_nc():
    if "nc" not in _CACHE:
        _CACHE["nc"] = _build()
    return _CACHE["nc"]


def kernel(**inputs) -> np.ndarray:
    from concourse.bass_utils import run_bass_kernel_spmd

    nc = _get_nc()
    seq = np.ascontiguousarray(np.asarray(inputs["sequences"], dtype=np.float32))
    common = {
        k: np.ascontiguousarray(np.asarray(inputs[k], dtype=np.float32))
        for k in ("Wq", "bq", "Wk", "bk", "Wv", "bv")
    }
    in_maps = [dict(common, x=seq[b]) for b in range(NCORES)]
    res = run_bass_kernel_spmd(nc, in_maps, list(range(NCORES)))
    return np.stack([res.results[b]["out"] for b in range(NCORES)], axis=0)


# revision 47
# speedup vs baseline: 2.0050x; 1.0108x over previous
"""Multi-head attention Trainium2 Bass kernel.

Problem: B=8, S=1024, D=768, H=12, head_dim=64; per-head block-diagonal QKV
projections + softmax attention (no 1/sqrt(hd) scaling).

Sharding: data-parallel over batch — one batch element per NeuronCore (8
cores). No collectives; host scatters inputs / gathers outputs.

Per-core dataflow (channel-on-partition "transposed" layouts; heads processed
in pairs p = (2p, 2p+1) matching 128-channel blocks of the embedding dim):
  x [S,D] --PE-transpose--> xT_r (f32r) / xT_bf (bf16)
  qT/kT = W.T @ xT + b      (f32r matmuls, pair row-tiled (0,0)/(64,0))
  v~ = x @ Wv | rank-1 bias update appends a ones column    ([t, 65] bf16)
  LT = K Q^T                ([t,s] layout, f32r); [128,1024] psum tiles,
                            bufs=2 so the next tile's matmuls overlap exp
  E^T = exp(LT)             (one ScalarE op per lt tile -> bf16)
  O = E @ v~ per (pair, s-tile): lhsT = E^T chunk (stationary), rhs = v~;
      psum [128(s), 130] = both heads; cols 64/129 = softmax denominators
  out = O * recip(denom)    (one batched DVE mul per group via 3-dim APs)

Scheduling: ScalarE is the bottleneck (~100us of exp); everything else is
interleaved into the exp ladder as filler so it never starves: transposes +
q/k proj of pair P ride in ladders < P, v-projections of pair p in ladder p,
attention-weighted-V of pair p-1 in ladder p, and the output DMA for head
pairs 0-4 of each s-tile overlaps the last ladder.
"""
import numpy as np

S = 1024
D = 768
H = 12
HD = 64
NPAIR = H // 2   # 6
NCORES = 8
ST = S // 128    # 8 s-tiles
TT = S // 128    # 8 t-tiles

_CACHE = {}


def _build():
    import contextlib
    import concourse.bacc as bacc
    import concourse.mybir as mybir
    import concourse.tile as tile
    from concourse.masks import make_identity

    f32 = mybir.dt.float32
    f32r = mybir.dt.float32r
    bf16 = mybir.dt.bfloat16
    Exp = mybir.ActivationFunctionType.Exp

    nc = bacc.Bacc("TRN2", target_bir_lowering=False, debug=False,
                   num_devices=NCORES)
    x = nc.declare_dram_parameter("x", [S, D], f32, isOutput=False)
    Wq = nc.declare_dram_parameter("Wq", [H, HD, HD], f32, isOutput=False)
    bq = nc.declare_dram_parameter("bq", [H, HD], f32, isOutput=False)
    Wk = nc.declare_dram_parameter("Wk", [H, HD, HD], f32, isOutput=False)
    bk = nc.declare_dram_parameter("bk", [H, HD], f32, isOutput=False)
    Wv = nc.declare_dram_parameter("Wv", [H, HD, HD], f32, isOutput=False)
    bv = nc.declare_dram_parameter("bv", [H, HD], f32, isOutput=False)
    out = nc.declare_dram_parameter("out", [S, D], f32, isOutput=True)

    with tile.TileContext(nc) as tc, contextlib.ExitStack() as ctx:
        singles = ctx.enter_context(tc.tile_pool(name="singles", bufs=1))
        per = ctx.enter_context(tc.tile_pool(name="per", bufs=1))
        qk_pool = ctx.enter_context(tc.tile_pool(name="qk", bufs=4))
        small_sb = ctx.enter_context(tc.tile_pool(name="small_sb", bufs=4))
        xload = ctx.enter_context(tc.tile_pool(name="xload", bufs=12))
        et_pool = ctx.enter_context(tc.tile_pool(name="et", bufs=36))
        # PSUM budget (8 banks): sp 2 + lt 2x2 + o 2x1 = 8
        sp_ps = ctx.enter_context(
            tc.tile_pool(name="sp_ps", bufs=2, space="PSUM"))
        lt_ps = ctx.enter_context(
            tc.tile_pool(name="lt_ps", bufs=2, space="PSUM"))
        o_ps = ctx.enter_context(
            tc.tile_pool(name="o_ps", bufs=2, space="PSUM"))

        # ---- persistent activations ----
        xT_r = [per.tile([128, S], f32r, tag=f"xT{i}", name=f"xT{i}")
                for i in range(6)]
        xT_bf = [per.tile([128, S], bf16, tag=f"xTb{i}", name=f"xTb{i}")
                 for i in range(6)]
        v_bf = [per.tile([128, TT, HD + 1], bf16, tag=f"v{h}", name=f"v{h}")
                for h in range(H)]
        staging = [per.tile([128, D], f32, tag=f"stg{i}", name=f"stg{i}")
                   for i in range(ST)]
        qT = {}
        kT = {}

        ident_f = singles.tile([128, 128], f32)
        make_identity(nc, ident_f)
        ones_bf = singles.tile([1, 128], bf16)
        nc.vector.memset(ones_bf, 1.0)
        # warm the ScalarE activation table (exp set) during the otherwise
        # idle lead-in so the ~1.3us table load is off the first-exp path
        warm = singles.tile([1, 1], f32, tag="warm", name="warm")
        nc.vector.memset(warm, 0.0)
        nc.scalar.activation(warm, warm, Exp)
        # warm the PE clock (HAM p-state) with throwaway matmuls while the
        # first x chunks stream in
        for _ in range(3):
            pw = o_ps.tile([128, 128], f32, tag="po", name="pw")
            nc.tensor.matmul(pw, ident_f, ident_f, start=True, stop=True)

        def emit_transposes(p):
            """x block p -> xT_r[p]: per-block [128,128] chunk DMAs feeding
            two PE transposes per psum tile + one DVE copyback."""
            csl = slice(p * 128, (p + 1) * 128)
            for st2 in range(ST // 2):
                tp = sp_ps.tile([128, 256], f32, tag="sps", name="tp")
                for j in range(2):
                    st = st2 * 2 + j
                    xc = xload.tile([128, 128], f32, tag="xf",
                                    name=f"xc{p}_{st}")
                    nc.sync.dma_start(
                        out=xc, in_=x[st * 128:(st + 1) * 128, csl])
                    nc.tensor.transpose(
                        tp[:, j * 128:(j + 1) * 128], xc, ident_f)
                nc.vector.tensor_copy(
                    xT_r[p][:, st2 * 256:(st2 + 1) * 256], tp)

        emit_transposes(0)

        def load_w_pair(w_dram, dtype, ncols, tag):
            raw = singles.tile([128, NPAIR, HD], f32, tag="wraw",
                               name=f"raw_{tag}")
            nc.sync.dma_start(
                out=raw[0:64, :, :],
                in_=w_dram[0:H:2, :, :].rearrange("h d e -> d h e"))
            nc.sync.dma_start(
                out=raw[64:128, :, :],
                in_=w_dram[1:H:2, :, :].rearrange("h d e -> d h e"))
            wt = singles.tile([128, NPAIR, ncols], dtype, tag=tag, name=tag)
            if ncols != HD:
                nc.vector.memset(wt, 0.0)
            nc.vector.tensor_copy(wt[:, :, 0:HD], raw)
            return wt

        def load_b_pair(b_dram, tag):
            bt = singles.tile([128, NPAIR], f32, tag=tag, name=tag)
            nc.sync.dma_start(out=bt[0:64, :],
                              in_=b_dram[0:H:2, :].rearrange("h e -> e h"))
            nc.sync.dma_start(out=bt[64:128, :],
                              in_=b_dram[1:H:2, :].rearrange("h e -> e h"))
            return bt

        wq_r = load_w_pair(Wq, f32r, HD, "wqr")
        wk_r = load_w_pair(Wk, f32r, HD, "wkr")
        bq_sb = load_b_pair(bq, "bqsb")
        bk_sb = load_b_pair(bk, "bksb")
        wv_bf = load_w_pair(Wv, bf16, HD + 1, "wvbf")
        bv_raw = singles.tile([1, H, HD], f32, tag="bvraw")
        nc.sync.dma_start(out=bv_raw, in_=bv[None, :, :])
        bv_bf = singles.tile([1, H, HD + 1], bf16)
        nc.vector.memset(bv_bf, 1.0)
        nc.vector.tensor_copy(bv_bf[:, :, 0:HD], bv_raw)

        def emit_proj_qk(p):
            qT[p] = qk_pool.tile([128, S], f32r, tag="qT", name=f"qT{p}")
            kT[p] = qk_pool.tile([128, S], f32r, tag="kT", name=f"kT{p}")
            for (wt, bt, dst) in ((wq_r, bq_sb, qT[p]), (wk_r, bk_sb, kT[p])):
                for sp in range(2):
                    sl = slice(sp * 512, (sp + 1) * 512)
                    psA = sp_ps.tile([64, 512], f32, tag="sps", name="psA")
                    psB = sp_ps.tile([64, 512], f32, tag="sps", name="psB")
                    nc.tensor.matmul(psA, wt[0:64, p, :], xT_r[p][0:64, sl],
                                     start=True, stop=True)
                    nc.tensor.matmul(psB, wt[64:128, p, :],
                                     xT_r[p][64:128, sl],
                                     start=True, stop=True)
                    nc.vector.tensor_scalar_add(
                        dst[0:64, sl], psA, bt[0:64, p:p + 1])
                    nc.vector.tensor_scalar_add(
                        dst[64:128, sl], psB, bt[64:128, p:p + 1])

        def emit_prep(p):
            emit_transposes(p)
            emit_proj_qk(p)

        def emit_v(p):
            """bf16 cast of block p + v projections for both heads."""
            nc.vector.tensor_copy(xT_bf[p], xT_r[p])
            for hh in range(2):
                h = 2 * p + hh
                base = 64 * hh
                for half in range(2):
                    pv = sp_ps.tile([128, 4, HD + 1], f32, tag="sps",
                                    name="pv")
                    for j in range(4):
                        tt = half * 4 + j
                        nc.tensor.matmul(
                            pv[:, j, :],
                            xT_bf[p][base:base + 64,
                                     tt * 128:(tt + 1) * 128],
                            wv_bf[base:base + 64, p, :],
                            start=True, stop=False)
                        nc.tensor.matmul(
                            pv[:, j, :], ones_bf, bv_bf[:, h, :],
                            start=False, stop=True)
                    nc.vector.tensor_copy(
                        v_bf[h][:, half * 4:(half + 1) * 4, :], pv)

        ET = {}

        def emit_av_group(p, st):
            """O for both heads of pair p at s-tile st: [128, 130] psum;
            cols 64/129 hold the softmax denominators. Runs at mid priority
            (above other filler, below the ladder) so the ET tiles of pair p
            release before pair p+1's ladder needs the slots."""
            _emit_av_group(p, st)

        def _emit_av_group(p, st):
            po = o_ps.tile([128, 2 * (HD + 1)], f32, tag="po", name="po")
            for hh in range(2):
                h = 2 * p + hh
                osl = slice(hh * (HD + 1), (hh + 1) * (HD + 1))
                for tt in range(TT):
                    nc.tensor.matmul(
                        po[:, osl],
                        ET[p][2 * tt + hh][:, st * 128:(st + 1) * 128],
                        v_bf[h][:, tt, :],
                        start=(tt == 0), stop=(tt == TT - 1))
            rc = small_sb.tile([128, 2], f32, tag="rc", name="rc")
            po3 = po.rearrange("a (h e) -> a h e", e=HD + 1)
            nc.vector.reciprocal(rc, po3[:, :, HD])
            nc.vector.tensor_tensor(
                out=staging[st].rearrange("a (h e) -> a h e", e=HD)[
                    :, 2 * p:2 * p + 2, :],
                in0=po3[:, :, 0:HD],
                in1=rc.rearrange("a (h o) -> a h o", o=1).to_broadcast(
                    (128, 2, HD)),
                op=mybir.AluOpType.mult)

        def emit_av_head(p, hh, st):
            """Single-head AV + normalize (used to drain the last pair)."""
            h = 2 * p + hh
            po = o_ps.tile([128, HD + 1], f32, tag="po", name="po")
            for tt in range(TT):
                nc.tensor.matmul(
                    po, ET[p][2 * tt + hh][:, st * 128:(st + 1) * 128],
                    v_bf[h][:, tt, :],
                    start=(tt == 0), stop=(tt == TT - 1))
            rc = small_sb.tile([128, 1], f32, tag="rc", name="rc")
            nc.vector.reciprocal(rc, po[:, HD:HD + 1])
            nc.vector.tensor_scalar_mul(
                staging[st][:, h * HD:(h + 1) * HD], po[:, 0:HD], rc)

        def emit_ladder(p, filler, hh_major=False):
            """LT + exp ladder for pair p; ET tile index = 2*tt + hh.
            hh_major orders all head-0 exps first so that head-0's AV can
            overlap the head-1 exps (used for the last pair)."""
            ET[p] = [None] * (2 * TT)
            if hh_major:
                units = [(tt, hh) for hh in range(2) for tt in range(TT)]
            else:
                units = [(tt, hh) for tt in range(TT) for hh in range(2)]
            for tt, hh in units:
                tsl = slice(tt * 128, (tt + 1) * 128)
                if True:
                    rsl = slice(hh * 64, hh * 64 + 64)
                    with tc.high_priority(offset=400):
                        lt = lt_ps.tile([128, 1024], f32, tag="lt",
                                        name="lt")
                        for sp in range(2):
                            ssl = slice(sp * 512, (sp + 1) * 512)
                            nc.tensor.matmul(lt[:, ssl], kT[p][rsl, tsl],
                                             qT[p][rsl, ssl],
                                             start=True, stop=True)
                        et = et_pool.tile([128, 1024], bf16, tag="et",
                                          name=f"et{p}_{2 * tt + hh}")
                        ET[p][2 * tt + hh] = et
                        nc.scalar.activation(et, lt, Exp)
                    if filler:
                        filler.pop(0)()
            while filler:
                filler.pop(0)()
            if p - 1 in ET:
                del ET[p - 1]

        emit_proj_qk(0)
        # filler plans per ladder (see module docstring)
        plans = {
            0: [lambda: emit_prep(1), lambda: emit_v(0),
                lambda: emit_prep(2), lambda: emit_prep(3)],
            1: [lambda: emit_prep(4), lambda: emit_v(1)],
            2: [lambda: emit_prep(5), lambda: emit_v(2)],
            3: [lambda: emit_v(3)],
            4: [lambda: emit_v(4)],
            5: [lambda: emit_v(5)],
        }
        for p in range(NPAIR):
            filler = list(plans[p])
            # delay AV fillers to mid-ladder: their matmuls wait on the
            # previous pair's last exp and would head-block the in-order PE
            # stream if scheduled early
            while len(filler) < 5:
                filler.append(lambda: None)
            if p >= 1:
                for st in range(ST):
                    filler.append(lambda q=p - 1, s=st: emit_av_group(q, s))
                    if p == NPAIR - 1:
                        # pairs 0-4 of this s-tile are final: overlap the
                        # bulk of the output writeback with the last ladder
                        filler.append(lambda s=st: nc.sync.dma_start(
                            out=out[s * 128:(s + 1) * 128, 0:640],
                            in_=staging[s][:, 0:640]))
            if p == NPAIR - 1:
                # last pair: head-0 exps first, then while head-1 exps run,
                # head-0's AV groups drain as trailing filler
                for st in range(ST):
                    filler.append(
                        lambda s=st: emit_av_head(NPAIR - 1, 0, s))
                emit_ladder(p, filler, hh_major=True)
            else:
                emit_ladder(p, filler)
        for st in range(ST):
            emit_av_head(NPAIR - 1, 1, st)
            nc.sync.dma_start(
                out=out[st * 128:(st + 1) * 128, 640:768],
                in_=staging[st][:, 640:768])

    nc.compile()
    return nc


def _get_nc():
    if "nc" not in _CACHE:
        _CACHE["nc"] = _build()
    return _CACHE["nc"]


def kernel(**inputs) -> np.ndarray:
    from concourse.bass_utils import run_bass_kernel_spmd

    nc = _get_nc()
    seq = np.ascontiguousarray(np.asarray(inputs["sequences"], dtype=np.float32))
    common = {
        k: np.ascontiguousarray(np.asarray(inputs[k], dtype=np.float32))
        for k in ("Wq", "bq", "Wk", "bk", "Wv", "bv")
    }
    in_maps = [dict(common, x=seq[b]) for b in range(NCORES)]
    res = run_bass_kernel_spmd(nc, in_maps, list(range(NCORES)))
    return np.stack([res.results[b]["out"] for b in range(NCORES)], axis=0)
